# revision 2
# baseline (speedup 1.0000x reference)
"""AttnBlock on 8 NeuronCores: fp8 boundary transfers, f32 on-device compute,
residual add on host. One batch sample per core via pmap."""

import numpy as np
import ml_dtypes

B, C, H, W = 8, 256, 112, 112
PATCH = 14
S = (H * W) // (PATCH * PATCH)  # 64
P = PATCH * PATCH  # 196
TG = PATCH * 4  # 56
A = TG * TG  # 3136
PW, GW = 0.75, 0.25
EPS = 1e-5
DSCALE = 128.0

FP8 = ml_dtypes.float8_e4m3


def _resize_matrix():
    n_out, n_in = H, TG
    xs = (np.arange(n_out, dtype=np.float64) + 0.5) / 2.0 - 0.5
    x0 = np.floor(xs).astype(np.int64)
    frac = xs - x0
    x0c = np.clip(x0, 0, n_in - 1)
    x1c = np.clip(x0 + 1, 0, n_in - 1)
    M = np.zeros((n_out, n_in), dtype=np.float64)
    M[np.arange(n_out), x0c] += 1.0 - frac
    M[np.arange(n_out), x1c] += frac
    return M.astype(np.float32)


_M_UP = _resize_matrix()
_fn = None


def _build():
    global _fn
    if _fn is not None:
        return _fn
    import jax
    import jax.numpy as jnp

    M_up = jnp.asarray(_M_UP)

    def sample(x8, gn_w, gn_b, wq, bq, wk, bk, wv, bv, w_proj):
        # x8: [C, H*W] fp8
        xf = x8.astype(jnp.float32)
        mu = jnp.mean(xf)
        var = jnp.mean((xf - mu) ** 2)
        xn = (xf - mu) * jax.lax.rsqrt(var + EPS)
        xn = xn * gn_w[:, None] + gn_b[:, None]

        q = wq @ xn + bq[:, None]
        k = wk @ xn + bk[:, None]
        v = wv @ xn + bv[:, None]

        qm = q.reshape(C * S, P)
        km = k.reshape(C * S, P)
        vm = v.reshape(C * S, P)
        att = (qm.T @ km) * np.float32((C * S) ** -0.5)
        att = jax.nn.softmax(att, axis=-1)
        h_patch = (vm @ att.T).reshape(C, H * W)

        def pool(t):
            return t.reshape(C, TG, 2, TG, 2).mean(axis=(2, 4)).reshape(C, A)

        qg = pool(q.reshape(C, H, W))
        kg = pool(k.reshape(C, H, W))
        vg = pool(v.reshape(C, H, W))
        attg = (qg.T @ kg) * np.float32(C ** -0.5)
        attg = jax.nn.softmax(attg, axis=-1)
        hg = (vg @ attg.T).reshape(C, TG, TG)
        hg = jnp.einsum("ij,cjk->cik", M_up, hg)
        hg = jnp.einsum("kj,cij->cik", M_up, hg)
        h_glob = hg.reshape(C, H * W)

        h = PW * h_patch + GW * h_glob
        delta = (w_proj @ h) * DSCALE
        return delta.astype(jnp.float8_e4m3)

    _fn = jax.pmap(sample, in_axes=(0,) + (None,) * 9,
                   devices=jax.devices()[:8])
    return _fn


def kernel(x, gn_w, gn_b, wq, bq, wk, bk, wv, bv, w_proj):
    fn = _build()
    x = np.asarray(x, np.float32)
    x8 = x.reshape(B, C, H * W).astype(FP8)
    d8 = fn(
        x8,
        np.asarray(gn_w, np.float32),
        np.asarray(gn_b, np.float32),
        np.asarray(wq, np.float32),
        np.asarray(bq, np.float32),
        np.asarray(wk, np.float32),
        np.asarray(bk, np.float32),
        np.asarray(wv, np.float32),
        np.asarray(bv, np.float32),
        np.asarray(w_proj, np.float32),
    )
    delta = np.asarray(d8).astype(np.float32)
    delta *= np.float32(1.0 / DSCALE)
    return x + delta.reshape(B, C, H, W)
